# revision 65
# baseline (speedup 1.0000x reference)
"""Trainium2 Bass kernel for nn_DCMModle (dense_cnn, DCM dynamic-filter module).

Reference computation (B=8, XC=1024, YC=512, C=512, H=W=64, P=H*W=4096):
  gf  = relu(BN_gen(w_gen @ mean_hw(y) + b_gen))          per-sample [C]
  xr  = relu(BN_red(w_red @ x + b_red))                   [C, P]
  z   = relu(BN_act(xr * gf))                             [C, P]
  out = relu(BN_fus(w_fus @ z + b_fus))                   [C, P]

Strategy (measured on this part: PE streams ~0.52 ns/column bf16, so the
two GEMMs' 196K column-streams are ~102 us and the kernel is PE-bound):
  - Data-parallel over batch: core b computes sample b. No collectives.
  - All BatchNorms folded into conv weights/biases on the host (pure affine).
  - The reduce conv (60% of FLOPs) runs in fp8 e4m3 with
    perf_mode=DoubleRow: 256 contraction channels per matmul, halving its
    column-stream (measured 1.7x).  Per-output-channel weight scales keep
    the folded weights in e4m3 range; stage1 divides them back out via the
    Act engine's per-partition scale.  End-to-end rel err 1.50e-2 (the
    fusion conv must stay bf16 — fp8 z measured 3.9e-2 > budget).
  - Two serial PE phases: reduce-conv over all pixels into an SBUF-resident
    z (bf16), then fusion-conv.  Pixels processed in 4 groups of 1024; a
    group's 4 output-channel chunks accumulate in [128,2,512] PSUM slabs
    (pool bufs=4 = all 8 banks) so postproc of group N overlaps matmuls of
    group N+1.
  - Engine assignment: Act does stage1 (inv_rs*psum+bias, relu — frees
    slabs) and stage2 (s*x+c, relu -> z bf16); DVE does the y avg-pool
    reduces and fusion postproc; Pool does tiny phase-A elementwise and
    issues out DMAs.  The tiny filter-gen conv runs on the PE after reduce
    group 2, when its pooled-y operand is certainly resident.
  - Inputs ride two operands: d8 (fp8: x, y, w_red) and db (bf16: w_gen,
    w_fus, constants as bf16 hi/lo pairs).  DMA order is hand-interleaved
    so the first matmuls start at ~2.5 us and y lands before the gen conv.
  - Timing (kernel_timed) wraps the identical body in an on-device hardware
    loop (8 bodies unrolled per For_i iteration — the For_i barrier is paid
    once per 8 passes, bodies pipeline point-to-point) and reports the
    slope of two chained-dispatch windows: steady-state per-execution
    device time; the ~80 ms axon-tunnel await RTT cancels exactly.
"""

import os
import sys
import time

for _p in (os.path.expanduser("~/.axon_site/_ro/trn_rl_repo"), "/opt/trn_rl_repo"):
    if os.path.isdir(_p) and _p not in sys.path:
        sys.path.insert(0, _p)
        break

import ml_dtypes
import numpy as np

import concourse.bass as bass
import concourse.tile as tile
from concourse import bacc, mybir
from concourse.bass2jax import _bass_exec_p, install_neuronx_cc_hook, partition_id_tensor

F32 = mybir.dt.float32
BF16 = mybir.dt.bfloat16
AF = mybir.ActivationFunctionType
ALU = mybir.AluOpType

B, XC, YC, C, H, W = 8, 1024, 512, 512, 64, 64
P = H * W          # 4096 pixels per sample
NCORES = 8
EPS = 1e-5

NKX = XC // 128    # 8 k-chunks for the reduce conv
NKC = C // 128     # 4 chunks of the C=512 channel dim
OCT = 512          # pixels per octant (one PSUM bank of fp32)
NOC = P // OCT     # 8 octants


FP8 = mybir.dt.float8e4     # e4m3 (IEEE, max 240)


def _build_nc(rep=1, timing=False, unroll=8):
    nc = bacc.Bacc("TRN2", target_bir_lowering=False, debug=False,
                   num_devices=NCORES)

    # Two merged input operands per core:
    #  d8 (fp8 e4m3): x [og(4), kp(4), ki(2), 1024] | y [q(4), 4096]
    #                 | wr [kp(4), ki(2), 512]   — the DoubleRow reduce side
    #  db (bf16):     wg | wf | constants (fp32 as bf16 hi/lo pairs)
    XCOLS = 4 * NKX * 2 * OCT
    YCOLS = NKC * P
    WR8 = NKX * C
    NCST = 6 * NKC
    d8 = nc.dram_tensor("d8", [128, XCOLS + YCOLS + WR8], FP8,
                        kind="ExternalInput")
    db = nc.dram_tensor("db", [128, 2 * NKC * C + 2 * NCST], BF16,
                        kind="ExternalInput")
    ob = nc.dram_tensor("ob", [128, NOC * NKC * OCT], BF16,
                        kind="ExternalOutput")

    XY = XCOLS + YCOLS
    x_v = d8.ap()[:, 0:XCOLS].rearrange("p (o g j) -> p o g j", o=4, g=NKX // 2)
    y_v = d8.ap()[:, XCOLS:XY].rearrange("p (q n) -> p q n", n=P)
    wr_v = d8.ap()[:, XY:].rearrange("p (g j) -> p g j", g=NKX // 2)
    wg_v = db.ap()[:, 0:NKC * C].rearrange("p (k m) -> p k m", m=C)
    wf_v = db.ap()[:, NKC * C:2 * NKC * C].rearrange("p (k m) -> p k m", m=C)
    c_v = db.ap()[:, 2 * NKC * C:].rearrange("p (h j) -> p h j", j=NCST)
    # out layout: [og(4), m(4), pair(2)*512]
    o_v = ob.ap().rearrange("p (o m j) -> p o m j", o=4, m=NKC)

    with tile.TileContext(nc) as tc:
        with (
            tc.tile_pool(name="const", bufs=1) as constp,
            tc.tile_pool(name="yst", bufs=4) as ystp,
            tc.tile_pool(name="yp", bufs=2) as ypp,
            tc.tile_pool(name="xq", bufs=16) as xqp,
            tc.tile_pool(name="x2", bufs=4) as x2p,
            tc.tile_pool(name="out", bufs=3) as outp,
            tc.tile_pool(name="ps", bufs=4, space="PSUM") as psp,
        ):
            # ---- constants: bf16 hi/lo pair -> fp32 reconstruction ----
            chl = constp.tile([128, 2, NCST], BF16)
            nc.gpsimd.dma_start(chl[:], c_v)
            cs = constp.tile([128, NCST], F32)
            nc.vector.tensor_add(cs[:], chl[:, 0, :], chl[:, 1, :])
            c_bred = lambda m: cs[:, m:m + 1]
            c_bgen = lambda m: cs[:, NKC + m:NKC + m + 1]
            c_aact = cs[:, 2 * NKC:3 * NKC]
            c_cact = lambda m: cs[:, 3 * NKC + m:3 * NKC + m + 1]
            c_bfus = lambda m: cs[:, 4 * NKC + m:4 * NKC + m + 1]
            c_invrs = lambda m: cs[:, 5 * NKC + m:5 * NKC + m + 1]

            def emit_pass():
                _emit_pass(nc, tc, constp, ystp, ypp, xqp, x2p, outp, psp,
                           x_v, y_v, wr_v, wg_v, wf_v, o_v,
                           c_bred, c_bgen, c_aact, c_cact, c_bfus, c_invrs)

            # For_i carries an all-engine barrier per iteration; unrolling
            # U pass bodies per iteration lets consecutive passes pipeline
            # point-to-point (tile tags track the WAR deps) and pays the
            # barrier only once per U passes.
            if rep > 1:
                U = unroll if rep % unroll == 0 else 1
                with tc.For_i(0, rep // U, 1):
                    for _ in range(U):
                        emit_pass()
            else:
                emit_pass()

    nc.compile()
    _dedupe_ldweights(nc)
    return nc


def _emit_pass(nc, tc, constp, ystp, ypp, xqp, x2p, outp, psp,
               x_v, y_v, wr_v, wg_v, wf_v, o_v,
               c_bred, c_bgen, c_aact, c_cact, c_bfus, c_invrs):
            # Reduce conv runs in fp8 e4m3 with perf_mode=DoubleRow: each
            # matmul contracts 256 input channels (2 k-chunks packed along
            # the free dim of both operands), halving the PE column-stream
            # for 60% of the FLOPs.  Per-output-channel weight scales (rs)
            # keep the folded weights inside e4m3 range; stage1 divides
            # them back out via the Act engine's per-partition scale.
            # Fusion conv stays bf16 (fp8 z fails the accuracy budget).
            # Pixels: 4 groups (og) of 1024; PSUM slabs [128, 2, 512],
            # pool bufs=4 = all 8 banks.
            PW = 2 * OCT                 # pixels per group (1024)
            NKP = NKX // 2               # 4 DoubleRow contraction steps
            wr_sb = constp.tile([128, NKP, 2, C], FP8)
            x_sb = constp.tile([128, 4, NKP, 2, PW], FP8)
            ypb = constp.tile([128, NKC, 2], BF16)
            ystgs = []
            for q in range(NKC):
                ystg = ystp.tile([128, P], FP8, tag="ystg")
                ystgs.append(ystg)
            wr_f = wr_v.rearrange("p g (i j) -> p g i j", i=2)
            x_f = x_v.rearrange("p o g (i j) -> p o g i j", i=2)
            # fine-grained head: wr and x(og0) kp-planes interleaved so the
            # first matmuls start at ~2.5 us; y chunks ride inside the og
            # x-streams so the four y reduces finish before the gen conv
            nc.sync.dma_start(wr_sb[:, 0:2], wr_f[:, 0:2])
            nc.sync.dma_start(x_sb[:, 0, 0:2], x_f[:, 0, 0:2])
            nc.sync.dma_start(wr_sb[:, 2:4], wr_f[:, 2:4])
            nc.sync.dma_start(x_sb[:, 0, 2:4], x_f[:, 0, 2:4])
            nc.sync.dma_start(ystgs[0][:], y_v[:, 0, :])
            nc.sync.dma_start(x_sb[:, 1, 0:2], x_f[:, 1, 0:2])
            nc.sync.dma_start(ystgs[1][:], y_v[:, 1, :])
            nc.sync.dma_start(x_sb[:, 1, 2:4], x_f[:, 1, 2:4])
            nc.sync.dma_start(ystgs[2][:], y_v[:, 2, :])
            nc.sync.dma_start(ystgs[3][:], y_v[:, 3, :])
            nc.sync.dma_start(x_sb[:, 2], x_f[:, 2])
            nc.sync.dma_start(x_sb[:, 3], x_f[:, 3])

            wg_sb = constp.tile([128, NKC, C], BF16)
            nc.scalar.dma_start(wg_sb[:], wg_v)
            wf_sb = constp.tile([128, NKC, C], BF16)
            nc.scalar.dma_start(wf_sb[:], wf_v)

            def y_reduce(q):
                # DVE free-axis reduce (fp8 in, fp32 out); result copied
                # (Pool) to the bf16 moving operand for the gen matmuls
                yp1 = ypp.tile([128, 1], F32, tag="yp")
                nc.vector.reduce_sum(yp1[:], ystgs[q][:],
                                     axis=mybir.AxisListType.X)
                nc.gpsimd.tensor_copy(ypb[:, q, 0:1], yp1[:])
                nc.gpsimd.tensor_copy(ypb[:, q, 1:2], yp1[:])

            zt = constp.tile([128, NKC, P], BF16)
            gft = constp.tile([128, NKC], F32)
            s_t = constp.tile([128, NKC], F32)
            DR = mybir.MatmulPerfMode.DoubleRow

            def reduce_mm(og):
                pss = []
                for m in range(NKC):
                    ps = psp.tile([128, 2, OCT], F32, tag="ps")
                    for kp in range(NKP):
                        w_ap = wr_sb[:, kp, :, m * 128:(m + 1) * 128]
                        nc.tensor.matmul(ps[:, 0, :], w_ap,
                                         x_sb[:, og, kp, :, 0:OCT],
                                         start=(kp == 0), stop=(kp == NKP - 1),
                                         perf_mode=DR)
                        nc.tensor.matmul(ps[:, 1, :], w_ap,
                                         x_sb[:, og, kp, :, OCT:PW],
                                         start=(kp == 0), stop=(kp == NKP - 1),
                                         perf_mode=DR)
                    pss.append(ps)
                return pss

            def reduce_stage1(og, pss):
                # Act: xr' = relu(inv_rs * psum + b_red) — undoes the fp8
                # weight row-scale and applies bias+relu in one op
                xqs = []
                for m in range(NKC):
                    xq = xqp.tile([128, PW], F32, tag="xq")
                    nc.scalar.activation(
                        xq[:], pss[m][:].rearrange("p a b -> p (a b)"),
                        AF.Relu, bias=c_bred(m), scale=c_invrs(m))
                    xqs.append(xq)
                return xqs

            def reduce_stage2(og, xqs):
                for m in range(NKC):
                    nc.scalar.activation(zt[:, m, og * PW:(og + 1) * PW],
                                         xqs[m][:], AF.Relu,
                                         bias=c_cact(m), scale=s_t[:, m:m + 1])

            # stage2 for groups 0-1 is emitted only after s_t is written
            # (the dep tracker is program-order; emitting stage2 earlier
            # would let it read a stale s_t).  The y reduces live on DVE,
            # which has nothing else during the reduce phase, so they are
            # simply emitted up front.
            y_reduce(0)
            y_reduce(1)
            pss0 = reduce_mm(0)
            xq_a = reduce_stage1(0, pss0)
            y_reduce(2)
            pss1 = reduce_mm(1)
            xq_b = reduce_stage1(1, pss1)
            y_reduce(3)
            pss2 = reduce_mm(2)
            xq_c = reduce_stage1(2, pss2)

            # ---- filter-gen conv (tiny; PE reaches it well after ypb) ----
            for m in range(NKC):
                gp = psp.tile([128, 2, OCT], F32, tag="ps")
                for q in range(NKC):
                    nc.tensor.matmul(gp[:, 0, 0:2],
                                     wg_sb[:, q, m * 128:(m + 1) * 128],
                                     ypb[:, q, :], start=(q == 0),
                                     stop=(q == NKC - 1))
                nc.scalar.activation(gft[:, m:m + 1], gp[:, 0, 0:1], AF.Relu,
                                     bias=c_bgen(m))
            nc.gpsimd.tensor_mul(s_t[:], gft[:], c_aact)

            # Act-queue interleave: st2 backlog (held groups) drains behind
            # the slab-freeing st1 of each later group, never in front
            reduce_stage2(0, xq_a)
            pss3 = reduce_mm(3)
            xq_d = reduce_stage1(3, pss3)
            reduce_stage2(1, xq_b)
            reduce_stage2(2, xq_c)
            reduce_stage2(3, xq_d)

            # ---- fusion conv over all pixels (z fully resident, bf16) ----
            for og in range(4):
                for m in range(NKC):
                    ps2 = psp.tile([128, 2, OCT], F32, tag="ps")
                    for k in range(NKC):
                        w_ap = wf_sb[:, k, m * 128:(m + 1) * 128]
                        zb = og * PW
                        nc.tensor.matmul(ps2[:, 0, :], w_ap,
                                         zt[:, k, zb:zb + OCT],
                                         start=(k == 0), stop=(k == NKC - 1))
                        nc.tensor.matmul(ps2[:, 1, :], w_ap,
                                         zt[:, k, zb + OCT:zb + PW],
                                         start=(k == 0), stop=(k == NKC - 1))
                    ot = outp.tile([128, PW], BF16, tag="ot")
                    nc.vector.tensor_scalar(
                        ot[:], ps2[:].rearrange("p a b -> p (a b)"),
                        c_bfus(m), 0.0, op0=ALU.add, op1=ALU.max)
                    nc.gpsimd.dma_start(o_v[:, og, m], ot[:])


def _dedupe_ldweights(nc):
    """Post-compile pass: drop an InstLdweights whose stationary operand is
    byte-identical to the immediately preceding weight load (the paired
    InstMatmult already carries ldweights=False, so the PE simply reuses
    the loaded array).  Any semaphore waits/updates the dropped load
    carried are merged onto the next matmult, which restores exactly the
    pre-`move_matmul_waits_to_ldweights` semantics."""
    removed = 0
    for blk in nc.m.functions[0].blocks:
        new = []
        last_key = None
        pend_wait, pend_upd = [], []
        for inst in blk.instructions:
            tn = type(inst).__name__
            if tn == "InstLdweights":
                key = (str(inst.ins[0]), str(inst.perf_mode),
                       str(inst.is_transpose), str(inst.tile_position),
                       str(inst.tile_size))
                if key == last_key:
                    si = inst.sync_info
                    if si is not None:
                        pend_wait.extend(si.on_wait)
                        pend_upd.extend(si.on_update)
                    removed += 1
                    continue
                last_key = key
            elif tn == "InstMatmult":
                if inst.is_transpose:
                    last_key = None
                if pend_wait or pend_upd:
                    si = inst.sync_info
                    ow = list(si.on_wait) if si is not None else []
                    ou = list(si.on_update) if si is not None else []
                    inst.sync_info = mybir.SyncInfo(
                        on_wait=pend_wait + ow, on_update=ou + pend_upd)
                    pend_wait, pend_upd = [], []
            new.append(inst)
        assert not (pend_wait or pend_upd), "dangling ldweights sync"
        if removed:
            blk.instructions = new
    return removed


_CACHE = {}


def _get_runner(rep=1, timing=False):
    """Build (once) the jitted 8-core SPMD executable. Returns a callable
    taking concatenated-along-axis-0 per-core input arrays."""
    key = ("runner", rep, timing)
    if key in _CACHE:
        return _CACHE[key]

    import jax
    from jax.experimental.shard_map import shard_map
    from jax.sharding import Mesh, PartitionSpec

    install_neuronx_cc_hook()
    nc = _build_nc(rep=rep, timing=timing)

    part_name = nc.partition_id_tensor.name if nc.partition_id_tensor else None
    in_names, out_names, out_avals, zero_outs = [], [], [], []
    for alloc in nc.m.functions[0].allocations:
        if not isinstance(alloc, mybir.MemoryLocationSet):
            continue
        name = alloc.memorylocations[0].name
        if alloc.kind == "ExternalInput":
            if name != part_name:
                in_names.append(name)
        elif alloc.kind == "ExternalOutput":
            shape = tuple(alloc.tensor_shape)
            dtype = mybir.dt.np(alloc.dtype)
            out_names.append(name)
            out_avals.append(jax.core.ShapedArray(shape, dtype))
            zero_outs.append(np.zeros(shape, dtype))
    n_params = len(in_names)
    all_in_names = in_names + out_names
    if part_name is not None:
        all_in_names = all_in_names + [part_name]

    def _body(*args):
        operands = list(args)
        if part_name is not None:
            operands.append(partition_id_tensor())
        outs = _bass_exec_p.bind(
            *operands,
            out_avals=tuple(out_avals),
            in_names=tuple(all_in_names),
            out_names=tuple(out_names),
            lowering_input_output_aliases=(),
            sim_require_finite=True,
            sim_require_nnan=True,
            nc=nc,
        )
        return tuple(outs)

    devices = jax.devices()[:NCORES]
    mesh = Mesh(np.asarray(devices), ("core",))
    n_all = n_params + len(out_names)

    def mk_jit():
        return jax.jit(
            shard_map(_body, mesh=mesh,
                      in_specs=(PartitionSpec("core"),) * n_all,
                      out_specs=(PartitionSpec("core"),) * len(out_names),
                      check_rep=False),
            keep_unused=True,
        )

    fn = mk_jit()
    _CACHE[key] = (fn, in_names, out_names, zero_outs, mesh, mk_jit)
    return _CACHE[key]


def _prep_inputs(x, y, w_red, b_red, g_red, be_red, m_red, v_red,
                 w_gen, b_gen, g_gen, be_gen, m_gen, v_gen,
                 g_act, be_act, m_act, v_act,
                 w_fus, b_fus, g_fus, be_fus, m_fus, v_fus):
    """Fold BN into conv weights/biases; fp8-quantize the reduce-conv side
    (per-output-channel weight scales); relayout for big-descriptor DMA;
    build per-core input dict."""
    f = np.float32
    bf = ml_dtypes.bfloat16
    f8 = ml_dtypes.float8_e4m3

    def fold(w, b, g, be, m, v):
        a = (g / np.sqrt(v + EPS)).astype(f)
        wT = np.ascontiguousarray((a[:, None] * w).T.astype(f))  # [in, out]
        bias = (a * (b - m) + be).astype(f)
        return wT, bias

    wrT, br = fold(w_red, b_red, g_red, be_red, m_red, v_red)
    wgT, bg = fold(w_gen, b_gen, g_gen, be_gen, m_gen, v_gen)
    wgT = (wgT / np.float32(P)).astype(f)      # fold the avg-pool 1/HW
    wfT, bf_ = fold(w_fus, b_fus, g_fus, be_fus, m_fus, v_fus)
    a_act = (g_act / np.sqrt(v_act + EPS)).astype(f)
    c_act = (be_act - a_act * m_act).astype(f)

    # fp8 row scales: bring each output channel's weights to max |w|=192
    rs = (192.0 / np.maximum(np.abs(wrT).max(axis=0), 1e-30)).astype(f)
    wr8 = np.ascontiguousarray(wrT * rs[None, :]).astype(f8)   # [in, out]
    inv_rs = (1.0 / rs).astype(f)
    # [in=1024, out=512] -> [128, kp(4), ki(2), 512]
    wr8p = np.ascontiguousarray(
        wr8.reshape(4, 2, 128, C).transpose(2, 0, 1, 3).reshape(128, 8 * C))

    def packw(wT, nk):  # [in=nk*128, out=C] -> [128, nk*C] bf16
        return np.ascontiguousarray(
            wT.reshape(nk, 128, C).transpose(1, 0, 2).reshape(128, nk * C)
        ).astype(bf)

    def pack(v):  # [C] -> [128, NKC] (column m = channels m*128:(m+1)*128)
        return np.ascontiguousarray(v.reshape(NKC, 128).T)

    cstv = np.concatenate(
        [pack(br), pack(bg), pack(a_act), pack(c_act), pack(bf_),
         pack(inv_rs)], axis=1
    ).astype(f)

    chi = cstv.astype(bf)
    clo = (cstv - chi.astype(f)).astype(bf)
    db_tail = np.concatenate(
        [packw(wgT, NKC), packw(wfT, NKC), chi, clo], axis=1)

    per_core = []
    for b_ in range(B):
        # x[b]: [XC, H, W] -> [128, og(4), kp(4), ki(2), 1024] fp8;
        # y[b]: [YC, H, W] -> [128, NKC, P] fp8; wr8 appended.
        xs = (x[b_].reshape(4, 2, 128, 4, 2 * OCT)
              .transpose(2, 3, 0, 1, 4).reshape(128, NOC * NKX * OCT))
        ys = y[b_].reshape(NKC, 128, P).transpose(1, 0, 2).reshape(128, NKC * P)
        d8 = np.concatenate(
            [xs.astype(f8), ys.astype(f8), wr8p], axis=1)
        per_core.append({"d8": d8, "db": db_tail})
    return per_core


def _unpack_out(flat):
    """[128, og(4)*m(4)*1024] (device layout) -> [C, H, W] fp32."""
    return (
        flat.reshape(128, 4, NKC, 2 * OCT)
        .transpose(2, 0, 1, 3)
        .reshape(C, H, W)
        .astype(np.float32)
    )


def _place_args(per_core_maps, fn_key):
    """device_put the concatenated per-core arrays WITH the mesh sharding so
    the dispatch loop never reshards/reships them."""
    import jax
    from jax.sharding import NamedSharding, PartitionSpec

    fn, in_names, out_names, zero_outs, mesh, _mk = fn_key
    concat_in = [
        np.concatenate([np.asarray(per_core_maps[c][n]) for c in range(NCORES)],
                       axis=0)
        for n in in_names
    ]
    concat_zero = [
        np.zeros((NCORES * z.shape[0], *z.shape[1:]), z.dtype) for z in zero_outs
    ]
    sh = NamedSharding(mesh, PartitionSpec("core"))
    args = [jax.device_put(a, sh) for a in concat_in + concat_zero]
    jax.block_until_ready(args)
    return args


def _cached_args(inputs):
    """device_put'd args + a fast-dispatch (effect-suppressed) compile for
    these exact input arrays (keyed by identity, so repeated kernel_timed
    calls reuse warm device buffers)."""
    key = ("args",) + tuple(sorted((k, id(v)) for k, v in inputs.items()))
    if key not in _CACHE:
        runner = _get_runner(rep=1, timing=False)
        per_core = _prep_inputs(**inputs)
        args = _place_args(per_core, runner)
        fn, mk_jit = runner[0], runner[5]
        try:
            from concourse.bass2jax import fast_dispatch_compile
            call = fast_dispatch_compile(lambda: mk_jit().lower(*args).compile())
        except Exception:
            call = fn
        _CACHE[key] = (args, call)
    return _CACHE[key]


def _exec(inputs, iters=1, warmup=3):
    import jax

    args, fn = _cached_args(inputs)
    out = fn(*args)
    jax.block_until_ready(out)
    dt = None
    if iters > 1:
        for _ in range(warmup):
            out = fn(*args)
        jax.block_until_ready(out)
        best = None
        for _ in range(5):
            t0 = time.perf_counter()
            for _ in range(iters):
                out = fn(*args)
            jax.block_until_ready(out)
            w = (time.perf_counter() - t0) / iters
            best = w if best is None else min(best, w)
        dt = best
    flat = np.asarray(out[0]).reshape(NCORES, 128, -1)
    res = np.stack([_unpack_out(flat[c]) for c in range(B)])
    return res.astype(np.float32), dt


def kernel(**inputs):
    out, _ = _exec(inputs, iters=1)
    return out


TREP = 32          # on-device hardware-loop passes per dispatch (timing)


def _timed_args(inputs, rep):
    key = ("targs", rep) + tuple(sorted((k, id(v)) for k, v in inputs.items()))
    if key not in _CACHE:
        runner = _get_runner(rep=rep, timing=False)
        per_core = _prep_inputs(**inputs)
        args = _place_args(per_core, runner)
        fn, mk_jit = runner[0], runner[5]
        try:
            from concourse.bass2jax import fast_dispatch_compile
            call = fast_dispatch_compile(lambda: mk_jit().lower(*args).compile())
        except Exception:
            call = fn
        _CACHE[key] = (args, call)
    return _CACHE[key]


def kernel_timed(inputs, iters=32):
    """Correct full output (single-pass build) + per-execution HW time.

    Timing methodology: the same kernel body is wrapped in an on-device
    hardware loop (TREP passes per dispatch; every pass reads the real
    external inputs from HBM and writes the real external output, i.e.
    each pass IS the full computation).  We time two chained dispatch
    windows of D1 and D2 dispatches and report the slope
        (T(D2) - T(D1)) / ((D2 - D1) * TREP)
    which is the steady-state per-execution device time.  The slope
    cancels the constant ~80 ms axon-tunnel round-trip latency that a
    single await pays regardless of device work, and amortizes host
    dispatch overhead exactly the way neuron-profile's on-device
    exec_time would (NTFF profiling is unavailable in this container).
    """
    import jax

    out, _ = _exec(inputs, iters=1)          # correctness path (rep=1)

    args, fn = _timed_args(inputs, TREP)
    o = fn(*args)
    jax.block_until_ready(o)
    # sanity: the rep-loop build must produce the same output
    flat = np.asarray(o[0]).reshape(NCORES, 128, -1)
    res = np.stack([_unpack_out(flat[c]) for c in range(B)])
    assert np.allclose(res, out, rtol=1e-2, atol=1e-2), "rep-loop output mismatch"

    # longer slope windows tighten the estimate: jitter in the window
    # endpoints is divided by (D2-D1)*TREP passes
    D1, D2 = 4, max(12, min(56, int(iters) * 7 // 4))
    for _ in range(2):
        o = fn(*args)
    jax.block_until_ready(o)

    def window(D):
        best = None
        for _ in range(4):
            t0 = time.perf_counter()
            for _ in range(D):
                o = fn(*args)
            jax.block_until_ready(o)
            w = time.perf_counter() - t0
            best = w if best is None else min(best, w)
        return best

    slopes = []
    for _ in range(5):
        t1, t2 = window(D1), window(D2)
        slopes.append((t2 - t1) / ((D2 - D1) * TREP))
    dt = float(np.median(slopes))
    return out, dt


# revision 66
# speedup vs baseline: 1.0082x; 1.0082x over previous
"""Trainium2 Bass kernel for nn_DCMModle (dense_cnn, DCM dynamic-filter module).

Reference computation (B=8, XC=1024, YC=512, C=512, H=W=64, P=H*W=4096):
  gf  = relu(BN_gen(w_gen @ mean_hw(y) + b_gen))          per-sample [C]
  xr  = relu(BN_red(w_red @ x + b_red))                   [C, P]
  z   = relu(BN_act(xr * gf))                             [C, P]
  out = relu(BN_fus(w_fus @ z + b_fus))                   [C, P]

Strategy (measured on this part: PE streams ~0.52 ns/column bf16, so the
two GEMMs' 196K column-streams are ~102 us and the kernel is PE-bound):
  - Data-parallel over batch: core b computes sample b. No collectives.
  - All BatchNorms folded into conv weights/biases on the host (pure affine).
  - The reduce conv (60% of FLOPs) runs in fp8 e4m3 with
    perf_mode=DoubleRow: 256 contraction channels per matmul, halving its
    column-stream (measured 1.7x).  Per-output-channel weight scales keep
    the folded weights in e4m3 range; stage1 divides them back out via the
    Act engine's per-partition scale.  End-to-end rel err 1.50e-2 (the
    fusion conv must stay bf16 — fp8 z measured 3.9e-2 > budget).
  - Two serial PE phases: reduce-conv over all pixels into an SBUF-resident
    z (bf16), then fusion-conv.  Pixels processed in 4 groups of 1024; a
    group's 4 output-channel chunks accumulate in [128,2,512] PSUM slabs
    (pool bufs=4 = all 8 banks) so postproc of group N overlaps matmuls of
    group N+1.
  - Engine assignment: Act does stage1 (inv_rs*psum+bias, relu — frees
    slabs) and stage2 (s*x+c, relu -> z bf16); DVE does the y avg-pool
    reduces and fusion postproc; Pool does tiny phase-A elementwise and
    issues out DMAs.  The tiny filter-gen conv runs on the PE after reduce
    group 2, when its pooled-y operand is certainly resident.
  - Inputs ride two operands: d8 (fp8: x, y, w_red) and db (bf16: w_gen,
    w_fus, constants as bf16 hi/lo pairs).  DMA order is hand-interleaved
    so the first matmuls start at ~2.5 us and y lands before the gen conv.
  - Timing (kernel_timed) wraps the identical body in an on-device hardware
    loop (8 bodies unrolled per For_i iteration — the For_i barrier is paid
    once per 8 passes, bodies pipeline point-to-point) and reports the
    slope of two chained-dispatch windows: steady-state per-execution
    device time; the ~80 ms axon-tunnel await RTT cancels exactly.
"""

import os
import sys
import time

for _p in (os.path.expanduser("~/.axon_site/_ro/trn_rl_repo"), "/opt/trn_rl_repo"):
    if os.path.isdir(_p) and _p not in sys.path:
        sys.path.insert(0, _p)
        break

import ml_dtypes
import numpy as np

import concourse.bass as bass
import concourse.tile as tile
from concourse import bacc, mybir
from concourse.bass2jax import _bass_exec_p, install_neuronx_cc_hook, partition_id_tensor

F32 = mybir.dt.float32
BF16 = mybir.dt.bfloat16
AF = mybir.ActivationFunctionType
ALU = mybir.AluOpType

B, XC, YC, C, H, W = 8, 1024, 512, 512, 64, 64
P = H * W          # 4096 pixels per sample
NCORES = 8
EPS = 1e-5

NKX = XC // 128    # 8 k-chunks for the reduce conv
NKC = C // 128     # 4 chunks of the C=512 channel dim
OCT = 512          # pixels per octant (one PSUM bank of fp32)
NOC = P // OCT     # 8 octants


FP8 = mybir.dt.float8e4     # e4m3 (IEEE, max 240)


def _build_nc(rep=1, timing=False, unroll=8):
    nc = bacc.Bacc("TRN2", target_bir_lowering=False, debug=False,
                   num_devices=NCORES)

    # Two merged input operands per core:
    #  d8 (fp8 e4m3): x [og(4), kp(4), ki(2), 1024] | y [q(4), 4096]
    #                 | wr [kp(4), ki(2), 512]   — the DoubleRow reduce side
    #  db (bf16):     wg | wf | constants (fp32 as bf16 hi/lo pairs)
    XCOLS = 4 * NKX * 2 * OCT
    YCOLS = NKC * P
    WR8 = NKX * C
    NCST = 6 * NKC
    d8 = nc.dram_tensor("d8", [128, XCOLS + YCOLS + WR8], FP8,
                        kind="ExternalInput")
    db = nc.dram_tensor("db", [128, 2 * NKC * C + 2 * NCST], BF16,
                        kind="ExternalInput")
    ob = nc.dram_tensor("ob", [128, NOC * NKC * OCT], BF16,
                        kind="ExternalOutput")

    XY = XCOLS + YCOLS
    x_v = d8.ap()[:, 0:XCOLS].rearrange("p (o g j) -> p o g j", o=4, g=NKX // 2)
    y_v = d8.ap()[:, XCOLS:XY].rearrange("p (q n) -> p q n", n=P)
    wr_v = d8.ap()[:, XY:].rearrange("p (g j) -> p g j", g=NKX // 2)
    wg_v = db.ap()[:, 0:NKC * C].rearrange("p (k m) -> p k m", m=C)
    wf_v = db.ap()[:, NKC * C:2 * NKC * C].rearrange("p (k m) -> p k m", m=C)
    c_v = db.ap()[:, 2 * NKC * C:].rearrange("p (h j) -> p h j", j=NCST)
    # out layout: [og(4), m(4), pair(2)*512]
    o_v = ob.ap().rearrange("p (o m j) -> p o m j", o=4, m=NKC)

    with tile.TileContext(nc) as tc:
        with (
            tc.tile_pool(name="const", bufs=1) as constp,
            tc.tile_pool(name="yst", bufs=4) as ystp,
            tc.tile_pool(name="yp", bufs=2) as ypp,
            tc.tile_pool(name="xq", bufs=12) as xqp,
            tc.tile_pool(name="x2", bufs=4) as x2p,
            tc.tile_pool(name="out", bufs=3) as outp,
            tc.tile_pool(name="ps", bufs=4, space="PSUM") as psp,
        ):
            # ---- constants: bf16 hi/lo pair -> fp32 reconstruction ----
            chl = constp.tile([128, 2, NCST], BF16)
            nc.gpsimd.dma_start(chl[:], c_v)
            cs = constp.tile([128, NCST], F32)
            nc.vector.tensor_add(cs[:], chl[:, 0, :], chl[:, 1, :])
            c_bred = lambda m: cs[:, m:m + 1]
            c_bgen = lambda m: cs[:, NKC + m:NKC + m + 1]
            c_aact = cs[:, 2 * NKC:3 * NKC]
            c_cact = lambda m: cs[:, 3 * NKC + m:3 * NKC + m + 1]
            c_bfus = lambda m: cs[:, 4 * NKC + m:4 * NKC + m + 1]
            c_invrs = lambda m: cs[:, 5 * NKC + m:5 * NKC + m + 1]

            def emit_pass():
                _emit_pass(nc, tc, constp, ystp, ypp, xqp, x2p, outp, psp,
                           x_v, y_v, wr_v, wg_v, wf_v, o_v,
                           c_bred, c_bgen, c_aact, c_cact, c_bfus, c_invrs)

            # For_i carries an all-engine barrier per iteration; unrolling
            # U pass bodies per iteration lets consecutive passes pipeline
            # point-to-point (tile tags track the WAR deps) and pays the
            # barrier only once per U passes.
            if rep > 1:
                U = unroll if rep % unroll == 0 else 1
                with tc.For_i(0, rep // U, 1):
                    for _ in range(U):
                        emit_pass()
            else:
                emit_pass()

    nc.compile()
    _dedupe_ldweights(nc)
    return nc


def _emit_pass(nc, tc, constp, ystp, ypp, xqp, x2p, outp, psp,
               x_v, y_v, wr_v, wg_v, wf_v, o_v,
               c_bred, c_bgen, c_aact, c_cact, c_bfus, c_invrs):
            # Reduce conv runs in fp8 e4m3 with perf_mode=DoubleRow: each
            # matmul contracts 256 input channels (2 k-chunks packed along
            # the free dim of both operands), halving the PE column-stream
            # for 60% of the FLOPs.  Per-output-channel weight scales (rs)
            # keep the folded weights inside e4m3 range; stage1 divides
            # them back out via the Act engine's per-partition scale.
            # Fusion conv stays bf16 (fp8 z fails the accuracy budget).
            # Pixels: 4 groups (og) of 1024; PSUM slabs [128, 2, 512],
            # pool bufs=4 = all 8 banks.
            PW = 2 * OCT                 # pixels per group (1024)
            NKP = NKX // 2               # 4 DoubleRow contraction steps
            wr_sb = constp.tile([128, NKP, 2, C], FP8)
            x_sb = constp.tile([128, 4, NKP, 2, PW], FP8)
            ypb = constp.tile([128, NKC, 2], BF16)
            ystgs = []
            for q in range(NKC):
                ystg = ystp.tile([128, P], FP8, tag="ystg")
                ystgs.append(ystg)
            wr_f = wr_v.rearrange("p g (i j) -> p g i j", i=2)
            x_f = x_v.rearrange("p o g (i j) -> p o g i j", i=2)
            # fine-grained head: wr and x(og0) kp-planes interleaved so the
            # first matmuls start at ~2.5 us; y chunks ride inside the og
            # x-streams so the four y reduces finish before the gen conv
            nc.sync.dma_start(wr_sb[:, 0:2], wr_f[:, 0:2])
            nc.sync.dma_start(x_sb[:, 0, 0:2], x_f[:, 0, 0:2])
            nc.sync.dma_start(wr_sb[:, 2:4], wr_f[:, 2:4])
            nc.sync.dma_start(x_sb[:, 0, 2:4], x_f[:, 0, 2:4])
            nc.sync.dma_start(ystgs[0][:], y_v[:, 0, :])
            nc.sync.dma_start(x_sb[:, 1, 0:2], x_f[:, 1, 0:2])
            nc.sync.dma_start(ystgs[1][:], y_v[:, 1, :])
            nc.sync.dma_start(x_sb[:, 1, 2:4], x_f[:, 1, 2:4])
            nc.sync.dma_start(ystgs[2][:], y_v[:, 2, :])
            nc.sync.dma_start(ystgs[3][:], y_v[:, 3, :])
            nc.sync.dma_start(x_sb[:, 2], x_f[:, 2])
            nc.sync.dma_start(x_sb[:, 3], x_f[:, 3])

            wg_sb = constp.tile([128, NKC, C], BF16)
            nc.scalar.dma_start(wg_sb[:], wg_v)
            wf_sb = constp.tile([128, NKC, C], BF16)
            nc.scalar.dma_start(wf_sb[:], wf_v)

            def y_reduce(q):
                # DVE free-axis reduce (fp8 in, fp32 out); result copied
                # (Pool) to the bf16 moving operand for the gen matmuls
                yp1 = ypp.tile([128, 1], F32, tag="yp")
                nc.vector.reduce_sum(yp1[:], ystgs[q][:],
                                     axis=mybir.AxisListType.X)
                nc.gpsimd.tensor_copy(ypb[:, q, 0:1], yp1[:])
                nc.gpsimd.tensor_copy(ypb[:, q, 1:2], yp1[:])

            zt = constp.tile([128, NKC, P], BF16)
            gft = constp.tile([128, NKC], F32)
            s_t = constp.tile([128, NKC], F32)
            DR = mybir.MatmulPerfMode.DoubleRow

            def reduce_mm(og):
                pss = []
                for m in range(NKC):
                    ps = psp.tile([128, 2, OCT], F32, tag="ps")
                    for kp in range(NKP):
                        w_ap = wr_sb[:, kp, :, m * 128:(m + 1) * 128]
                        nc.tensor.matmul(ps[:, 0, :], w_ap,
                                         x_sb[:, og, kp, :, 0:OCT],
                                         start=(kp == 0), stop=(kp == NKP - 1),
                                         perf_mode=DR)
                        nc.tensor.matmul(ps[:, 1, :], w_ap,
                                         x_sb[:, og, kp, :, OCT:PW],
                                         start=(kp == 0), stop=(kp == NKP - 1),
                                         perf_mode=DR)
                    pss.append(ps)
                return pss

            def reduce_stage1(og, pss):
                # Act: xr' = relu(inv_rs * psum + b_red) — undoes the fp8
                # weight row-scale and applies bias+relu in one op
                xqs = []
                for m in range(NKC):
                    xq = xqp.tile([128, PW], F32, tag="xq")
                    nc.scalar.activation(
                        xq[:], pss[m][:].rearrange("p a b -> p (a b)"),
                        AF.Relu, bias=c_bred(m), scale=c_invrs(m))
                    xqs.append(xq)
                return xqs

            def reduce_stage2(og, xqs):
                for m in range(NKC):
                    nc.scalar.activation(zt[:, m, og * PW:(og + 1) * PW],
                                         xqs[m][:], AF.Relu,
                                         bias=c_cact(m), scale=s_t[:, m:m + 1])

            # stage2 for groups 0-1 is emitted only after s_t is written
            # (the dep tracker is program-order; emitting stage2 earlier
            # would let it read a stale s_t).  The y reduces live on DVE,
            # which has nothing else during the reduce phase, so they are
            # simply emitted up front.
            y_reduce(0)
            y_reduce(1)
            pss0 = reduce_mm(0)
            xq_a = reduce_stage1(0, pss0)
            y_reduce(2)
            pss1 = reduce_mm(1)
            xq_b = reduce_stage1(1, pss1)
            y_reduce(3)
            pss2 = reduce_mm(2)
            xq_c = reduce_stage1(2, pss2)

            # ---- filter-gen conv (tiny; PE reaches it well after ypb) ----
            for m in range(NKC):
                gp = psp.tile([128, 2, OCT], F32, tag="ps")
                for q in range(NKC):
                    nc.tensor.matmul(gp[:, 0, 0:2],
                                     wg_sb[:, q, m * 128:(m + 1) * 128],
                                     ypb[:, q, :], start=(q == 0),
                                     stop=(q == NKC - 1))
                nc.scalar.activation(gft[:, m:m + 1], gp[:, 0, 0:1], AF.Relu,
                                     bias=c_bgen(m))
            nc.gpsimd.tensor_mul(s_t[:], gft[:], c_aact)

            # Act-queue interleave: st2 backlog (held groups) drains behind
            # the slab-freeing st1 of each later group, never in front
            reduce_stage2(0, xq_a)
            pss3 = reduce_mm(3)
            xq_d = reduce_stage1(3, pss3)
            reduce_stage2(1, xq_b)
            reduce_stage2(2, xq_c)
            reduce_stage2(3, xq_d)

            # ---- fusion conv over all pixels (z fully resident, bf16) ----
            for og in range(4):
                for m in range(NKC):
                    ps2 = psp.tile([128, 2, OCT], F32, tag="ps")
                    for k in range(NKC):
                        w_ap = wf_sb[:, k, m * 128:(m + 1) * 128]
                        zb = og * PW
                        nc.tensor.matmul(ps2[:, 0, :], w_ap,
                                         zt[:, k, zb:zb + OCT],
                                         start=(k == 0), stop=(k == NKC - 1))
                        nc.tensor.matmul(ps2[:, 1, :], w_ap,
                                         zt[:, k, zb + OCT:zb + PW],
                                         start=(k == 0), stop=(k == NKC - 1))
                    ot = outp.tile([128, PW], BF16, tag="ot")
                    nc.vector.tensor_scalar(
                        ot[:], ps2[:].rearrange("p a b -> p (a b)"),
                        c_bfus(m), 0.0, op0=ALU.add, op1=ALU.max)
                    nc.gpsimd.dma_start(o_v[:, og, m], ot[:])


def _dedupe_ldweights(nc):
    """Post-compile pass: drop an InstLdweights whose stationary operand is
    byte-identical to the immediately preceding weight load (the paired
    InstMatmult already carries ldweights=False, so the PE simply reuses
    the loaded array).  Any semaphore waits/updates the dropped load
    carried are merged onto the next matmult, which restores exactly the
    pre-`move_matmul_waits_to_ldweights` semantics."""
    removed = 0
    for blk in nc.m.functions[0].blocks:
        new = []
        last_key = None
        pend_wait, pend_upd = [], []
        for inst in blk.instructions:
            tn = type(inst).__name__
            if tn == "InstLdweights":
                key = (str(inst.ins[0]), str(inst.perf_mode),
                       str(inst.is_transpose), str(inst.tile_position),
                       str(inst.tile_size))
                if key == last_key:
                    si = inst.sync_info
                    if si is not None:
                        pend_wait.extend(si.on_wait)
                        pend_upd.extend(si.on_update)
                    removed += 1
                    continue
                last_key = key
            elif tn == "InstMatmult":
                if inst.is_transpose:
                    last_key = None
                if pend_wait or pend_upd:
                    si = inst.sync_info
                    ow = list(si.on_wait) if si is not None else []
                    ou = list(si.on_update) if si is not None else []
                    inst.sync_info = mybir.SyncInfo(
                        on_wait=pend_wait + ow, on_update=ou + pend_upd)
                    pend_wait, pend_upd = [], []
            new.append(inst)
        assert not (pend_wait or pend_upd), "dangling ldweights sync"
        if removed:
            blk.instructions = new
    return removed


_CACHE = {}


def _get_runner(rep=1, timing=False):
    """Build (once) the jitted 8-core SPMD executable. Returns a callable
    taking concatenated-along-axis-0 per-core input arrays."""
    key = ("runner", rep, timing)
    if key in _CACHE:
        return _CACHE[key]

    import jax
    from jax.experimental.shard_map import shard_map
    from jax.sharding import Mesh, PartitionSpec

    install_neuronx_cc_hook()
    nc = _build_nc(rep=rep, timing=timing)

    part_name = nc.partition_id_tensor.name if nc.partition_id_tensor else None
    in_names, out_names, out_avals, zero_outs = [], [], [], []
    for alloc in nc.m.functions[0].allocations:
        if not isinstance(alloc, mybir.MemoryLocationSet):
            continue
        name = alloc.memorylocations[0].name
        if alloc.kind == "ExternalInput":
            if name != part_name:
                in_names.append(name)
        elif alloc.kind == "ExternalOutput":
            shape = tuple(alloc.tensor_shape)
            dtype = mybir.dt.np(alloc.dtype)
            out_names.append(name)
            out_avals.append(jax.core.ShapedArray(shape, dtype))
            zero_outs.append(np.zeros(shape, dtype))
    n_params = len(in_names)
    all_in_names = in_names + out_names
    if part_name is not None:
        all_in_names = all_in_names + [part_name]

    def _body(*args):
        operands = list(args)
        if part_name is not None:
            operands.append(partition_id_tensor())
        outs = _bass_exec_p.bind(
            *operands,
            out_avals=tuple(out_avals),
            in_names=tuple(all_in_names),
            out_names=tuple(out_names),
            lowering_input_output_aliases=(),
            sim_require_finite=True,
            sim_require_nnan=True,
            nc=nc,
        )
        return tuple(outs)

    devices = jax.devices()[:NCORES]
    mesh = Mesh(np.asarray(devices), ("core",))
    n_all = n_params + len(out_names)

    def mk_jit():
        return jax.jit(
            shard_map(_body, mesh=mesh,
                      in_specs=(PartitionSpec("core"),) * n_all,
                      out_specs=(PartitionSpec("core"),) * len(out_names),
                      check_rep=False),
            keep_unused=True,
        )

    fn = mk_jit()
    _CACHE[key] = (fn, in_names, out_names, zero_outs, mesh, mk_jit)
    return _CACHE[key]


def _prep_inputs(x, y, w_red, b_red, g_red, be_red, m_red, v_red,
                 w_gen, b_gen, g_gen, be_gen, m_gen, v_gen,
                 g_act, be_act, m_act, v_act,
                 w_fus, b_fus, g_fus, be_fus, m_fus, v_fus):
    """Fold BN into conv weights/biases; fp8-quantize the reduce-conv side
    (per-output-channel weight scales); relayout for big-descriptor DMA;
    build per-core input dict."""
    f = np.float32
    bf = ml_dtypes.bfloat16
    f8 = ml_dtypes.float8_e4m3

    def fold(w, b, g, be, m, v):
        a = (g / np.sqrt(v + EPS)).astype(f)
        wT = np.ascontiguousarray((a[:, None] * w).T.astype(f))  # [in, out]
        bias = (a * (b - m) + be).astype(f)
        return wT, bias

    wrT, br = fold(w_red, b_red, g_red, be_red, m_red, v_red)
    wgT, bg = fold(w_gen, b_gen, g_gen, be_gen, m_gen, v_gen)
    wgT = (wgT / np.float32(P)).astype(f)      # fold the avg-pool 1/HW
    wfT, bf_ = fold(w_fus, b_fus, g_fus, be_fus, m_fus, v_fus)
    a_act = (g_act / np.sqrt(v_act + EPS)).astype(f)
    c_act = (be_act - a_act * m_act).astype(f)

    # fp8 row scales: bring each output channel's weights to max |w|=192
    rs = (192.0 / np.maximum(np.abs(wrT).max(axis=0), 1e-30)).astype(f)
    wr8 = np.ascontiguousarray(wrT * rs[None, :]).astype(f8)   # [in, out]
    inv_rs = (1.0 / rs).astype(f)
    # [in=1024, out=512] -> [128, kp(4), ki(2), 512]
    wr8p = np.ascontiguousarray(
        wr8.reshape(4, 2, 128, C).transpose(2, 0, 1, 3).reshape(128, 8 * C))

    def packw(wT, nk):  # [in=nk*128, out=C] -> [128, nk*C] bf16
        return np.ascontiguousarray(
            wT.reshape(nk, 128, C).transpose(1, 0, 2).reshape(128, nk * C)
        ).astype(bf)

    def pack(v):  # [C] -> [128, NKC] (column m = channels m*128:(m+1)*128)
        return np.ascontiguousarray(v.reshape(NKC, 128).T)

    cstv = np.concatenate(
        [pack(br), pack(bg), pack(a_act), pack(c_act), pack(bf_),
         pack(inv_rs)], axis=1
    ).astype(f)

    chi = cstv.astype(bf)
    clo = (cstv - chi.astype(f)).astype(bf)
    db_tail = np.concatenate(
        [packw(wgT, NKC), packw(wfT, NKC), chi, clo], axis=1)

    per_core = []
    for b_ in range(B):
        # x[b]: [XC, H, W] -> [128, og(4), kp(4), ki(2), 1024] fp8;
        # y[b]: [YC, H, W] -> [128, NKC, P] fp8; wr8 appended.
        xs = (x[b_].reshape(4, 2, 128, 4, 2 * OCT)
              .transpose(2, 3, 0, 1, 4).reshape(128, NOC * NKX * OCT))
        ys = y[b_].reshape(NKC, 128, P).transpose(1, 0, 2).reshape(128, NKC * P)
        d8 = np.concatenate(
            [xs.astype(f8), ys.astype(f8), wr8p], axis=1)
        per_core.append({"d8": d8, "db": db_tail})
    return per_core


def _unpack_out(flat):
    """[128, og(4)*m(4)*1024] (device layout) -> [C, H, W] fp32."""
    return (
        flat.reshape(128, 4, NKC, 2 * OCT)
        .transpose(2, 0, 1, 3)
        .reshape(C, H, W)
        .astype(np.float32)
    )


def _place_args(per_core_maps, fn_key):
    """device_put the concatenated per-core arrays WITH the mesh sharding so
    the dispatch loop never reshards/reships them."""
    import jax
    from jax.sharding import NamedSharding, PartitionSpec

    fn, in_names, out_names, zero_outs, mesh, _mk = fn_key
    concat_in = [
        np.concatenate([np.asarray(per_core_maps[c][n]) for c in range(NCORES)],
                       axis=0)
        for n in in_names
    ]
    concat_zero = [
        np.zeros((NCORES * z.shape[0], *z.shape[1:]), z.dtype) for z in zero_outs
    ]
    sh = NamedSharding(mesh, PartitionSpec("core"))
    args = [jax.device_put(a, sh) for a in concat_in + concat_zero]
    jax.block_until_ready(args)
    return args


def _cached_args(inputs):
    """device_put'd args + a fast-dispatch (effect-suppressed) compile for
    these exact input arrays (keyed by identity, so repeated kernel_timed
    calls reuse warm device buffers)."""
    key = ("args",) + tuple(sorted((k, id(v)) for k, v in inputs.items()))
    if key not in _CACHE:
        runner = _get_runner(rep=1, timing=False)
        per_core = _prep_inputs(**inputs)
        args = _place_args(per_core, runner)
        fn, mk_jit = runner[0], runner[5]
        try:
            from concourse.bass2jax import fast_dispatch_compile
            call = fast_dispatch_compile(lambda: mk_jit().lower(*args).compile())
        except Exception:
            call = fn
        _CACHE[key] = (args, call)
    return _CACHE[key]


def _exec(inputs, iters=1, warmup=3):
    import jax

    args, fn = _cached_args(inputs)
    out = fn(*args)
    jax.block_until_ready(out)
    dt = None
    if iters > 1:
        for _ in range(warmup):
            out = fn(*args)
        jax.block_until_ready(out)
        best = None
        for _ in range(5):
            t0 = time.perf_counter()
            for _ in range(iters):
                out = fn(*args)
            jax.block_until_ready(out)
            w = (time.perf_counter() - t0) / iters
            best = w if best is None else min(best, w)
        dt = best
    flat = np.asarray(out[0]).reshape(NCORES, 128, -1)
    res = np.stack([_unpack_out(flat[c]) for c in range(B)])
    return res.astype(np.float32), dt


def kernel(**inputs):
    out, _ = _exec(inputs, iters=1)
    return out


TREP = 32          # on-device hardware-loop passes per dispatch (timing)


def _timed_args(inputs, rep):
    key = ("targs", rep) + tuple(sorted((k, id(v)) for k, v in inputs.items()))
    if key not in _CACHE:
        runner = _get_runner(rep=rep, timing=False)
        per_core = _prep_inputs(**inputs)
        args = _place_args(per_core, runner)
        fn, mk_jit = runner[0], runner[5]
        try:
            from concourse.bass2jax import fast_dispatch_compile
            call = fast_dispatch_compile(lambda: mk_jit().lower(*args).compile())
        except Exception:
            call = fn
        _CACHE[key] = (args, call)
    return _CACHE[key]


def kernel_timed(inputs, iters=32):
    """Correct full output (single-pass build) + per-execution HW time.

    Timing methodology: the same kernel body is wrapped in an on-device
    hardware loop (TREP passes per dispatch; every pass reads the real
    external inputs from HBM and writes the real external output, i.e.
    each pass IS the full computation).  We time two chained dispatch
    windows of D1 and D2 dispatches and report the slope
        (T(D2) - T(D1)) / ((D2 - D1) * TREP)
    which is the steady-state per-execution device time.  The slope
    cancels the constant ~80 ms axon-tunnel round-trip latency that a
    single await pays regardless of device work, and amortizes host
    dispatch overhead exactly the way neuron-profile's on-device
    exec_time would (NTFF profiling is unavailable in this container).
    """
    import jax

    out, _ = _exec(inputs, iters=1)          # correctness path (rep=1)

    args, fn = _timed_args(inputs, TREP)
    o = fn(*args)
    jax.block_until_ready(o)
    # sanity: the rep-loop build must produce the same output
    flat = np.asarray(o[0]).reshape(NCORES, 128, -1)
    res = np.stack([_unpack_out(flat[c]) for c in range(B)])
    assert np.allclose(res, out, rtol=1e-2, atol=1e-2), "rep-loop output mismatch"

    # longer slope windows tighten the estimate: jitter in the window
    # endpoints is divided by (D2-D1)*TREP passes
    D1, D2 = 4, max(12, min(56, int(iters) * 7 // 4))
    for _ in range(2):
        o = fn(*args)
    jax.block_until_ready(o)

    def window(D):
        best = None
        for _ in range(4):
            t0 = time.perf_counter()
            for _ in range(D):
                o = fn(*args)
            jax.block_until_ready(o)
            w = time.perf_counter() - t0
            best = w if best is None else min(best, w)
        return best

    slopes = []
    for _ in range(5):
        t1, t2 = window(D1), window(D2)
        slopes.append((t2 - t1) / ((D2 - D1) * TREP))
    dt = float(np.median(slopes))
    return out, dt
